# revision 46
# baseline (speedup 1.0000x reference)
"""Trainium2 Bass kernel for a 3-block GPT (B=2,T=2048,E=1024,H=16,V=32000).

Sharding: sequence-parallel over 8 cores (512 tokens each, weights replicated).
Per layer: QKV local, group-local AllGather of K^T and V (per-batch groups
[[0-3],[4-7]] so the SPMD program is identical on every core), attention in
scores-transposed layout (softmax denominator via ones-matmul), proj/FFN with
weights stationary, LayerNorm in transposed layout via ones-matmul partition
reductions. lm_head produces logits^T [V, 512] per core; host reassembles.
Per-core differences (token ids, causal masks) enter via input data only.
"""

import numpy as np
import ml_dtypes
from contextlib import ExitStack

import concourse.bass as bass
import concourse.mybir as mybir
import concourse.tile as tile
from concourse import bacc
from concourse.masks import make_identity
from concourse import bass_utils

# model dims (hardcoded; harness contract)
B, T, E, H, V = 2, 2048, 1024, 16, 32000
HD, L = 64, 3
NC = 8
S = (B * T) // NC        # 512 tokens per core
CH = NC // B             # 4 chunks (cores) per batch
FF = 4 * E               # 4096
EPS = 1e-5
P = 128
NE = E // P              # 8 e-tiles
NHP = H // 2             # 8 head pairs
NSB = T // P             # 16 key blocks per batch
NVT = V // P             # 250 vocab tiles
NFT = FF // P            # 32 f-tiles
F32 = mybir.dt.float32
BF16 = mybir.dt.bfloat16
FP8 = mybir.dt.float8e4
I32 = mybir.dt.int32
AF = mybir.ActivationFunctionType
OP = mybir.AluOpType


def _ln_tiles(nc, tc, pools, src, out, g_t, b_t, ones_f, eps_t):
    """LayerNorm over E (partition axis across the 8 [128,S] tiles of src).

    src/out: lists of 8 SBUF tiles [128, S] (xT layout). g_t/b_t: [128, NE]
    param tiles; gcol/bcol: column index. Stats via ones-matmul partition
    reduction broadcast to all 128 partitions."""
    pacc, tp = pools["pacc"], pools["tp"]
    psm = pacc.tile([P, S], F32, tag="ps_a", name="ln_ps", bufs=6)
    pss = pacc.tile([P, S], F32, tag="ps_a", name="ln_ps", bufs=6)
    for e in range(NE):
        xc = tp.tile([P, S], BF16, tag="ln_xc", name="ln_xc", bufs=2)
        nc.vector.tensor_copy(xc[:], src[e][:])
        nc.tensor.matmul(psm[:], lhsT=ones_f[:], rhs=xc[:],
                         start=(e == 0), stop=(e == NE - 1), skip_group_check=True)
        sq = tp.tile([P, S], BF16, tag="ln_sq", name="ln_sq", bufs=2)
        nc.scalar.square(sq[:], src[e][:])
        nc.tensor.matmul(pss[:], lhsT=ones_f[:], rhs=sq[:],
                         start=(e == 0), stop=(e == NE - 1), skip_group_check=True)
    mean = tp.tile([P, S], F32, tag="ln_mean", name="ln_mean")
    nc.scalar.mul(mean[:], psm[:], 1.0 / E)
    msq = tp.tile([P, S], F32, tag="ln_msq", name="ln_msq")
    nc.scalar.square(msq[:], mean[:])
    var = tp.tile([P, S], F32, tag="ln_var", name="ln_var")
    nc.vector.tensor_scalar(var[:], pss[:], 1.0 / E, None, OP.mult)
    nc.vector.tensor_tensor(out=var[:], in0=var[:], in1=msq[:], op=OP.subtract)
    std = tp.tile([P, S], F32, tag="ln_std", name="ln_std")
    nc.scalar.activation(std[:], var[:], AF.Sqrt, bias=eps_t[:])
    rstd = tp.tile([P, S], F32, tag="ln_rstd", name="ln_rstd")
    nc.vector.reciprocal(rstd[:], std[:])
    for e in range(NE):
        t = tp.tile([P, S], F32, tag="ln_t", name="ln_t", bufs=2)
        nc.vector.tensor_tensor(out=t[:], in0=src[e][:], in1=mean[:], op=OP.subtract)
        nc.vector.tensor_tensor(out=t[:], in0=t[:], in1=rstd[:], op=OP.mult)
        nc.vector.tensor_scalar(out[e][:], t[:], g_t[:, e:e + 1],
                                b_t[:, e:e + 1], OP.mult, OP.add)


def build_program():
    nc = bacc.Bacc("TRN2", target_bir_lowering=False, debug=False, num_devices=NC)

    # ---- DRAM I/O ----
    d_wqr = nc.dram_tensor("wqr", [L, NE // 2, P, 2, NE, P], BF16, kind="ExternalInput")
    d_wkr = nc.dram_tensor("wkr", [L, NE // 2, P, 2, NE, P], BF16, kind="ExternalInput")
    d_wvf = nc.dram_tensor("wvf", [L, 2, P, NE, 512], BF16, kind="ExternalInput")
    d_wor = nc.dram_tensor("wor", [L, NE // 2, P, 2, NE, P], BF16, kind="ExternalInput")
    d_w1r = nc.dram_tensor("w1r", [L, NFT // 2, P, 2, NE, P], BF16, kind="ExternalInput")
    d_w2r = nc.dram_tensor("w2r", [L, NE, P, NFT, P], BF16, kind="ExternalInput")
    d_wlm = nc.dram_tensor("wlmr", [NVT // 2, P, 2, NE, P], BF16, kind="ExternalInput")
    d_emb = nc.dram_tensor("emb", [V, E], F32, kind="ExternalInput")
    d_idx = nc.dram_tensor("idx", [S], I32, kind="ExternalInput")
    d_msk = nc.dram_tensor("maskp", [NSB, P, S], BF16, kind="ExternalInput")
    d_ln1g = nc.dram_tensor("ln1g", [L, E], F32, kind="ExternalInput")
    d_ln1b = nc.dram_tensor("ln1b", [L, E], F32, kind="ExternalInput")
    d_ln2g = nc.dram_tensor("ln2g", [L, E], F32, kind="ExternalInput")
    d_ln2b = nc.dram_tensor("ln2b", [L, E], F32, kind="ExternalInput")
    d_bo = nc.dram_tensor("bo", [L, E], F32, kind="ExternalInput")
    d_b1 = nc.dram_tensor("b1", [L, FF], F32, kind="ExternalInput")
    d_b2 = nc.dram_tensor("b2", [L, E], F32, kind="ExternalInput")
    d_lnfg = nc.dram_tensor("lnfg", [E], F32, kind="ExternalInput")
    d_lnfb = nc.dram_tensor("lnfb", [E], F32, kind="ExternalInput")
    d_blm = nc.dram_tensor("blm", [V], F32, kind="ExternalInput")
    d_out = nc.dram_tensor("logt", [V, S], F32, kind="ExternalOutput")

    groups = [[0, 1, 2, 3], [4, 5, 6, 7]]

    with ExitStack() as ctx:
        tc = ctx.enter_context(tile.TileContext(nc, num_cores=NC))
        const = ctx.enter_context(tc.tile_pool(name="const", bufs=1))
        pp_x = ctx.enter_context(tc.tile_pool(name="xres", bufs=1))
        pp_sum = ctx.enter_context(tc.tile_pool(name="xsum", bufs=1))
        pp_msk = ctx.enter_context(tc.tile_pool(name="masks", bufs=1))
        tp = ctx.enter_context(tc.tile_pool(name="tp", bufs=1))
        wp = ctx.enter_context(tc.tile_pool(name="wstream", bufs=1))
        pacc = ctx.enter_context(tc.tile_pool(name="pacc", bufs=1, space="PSUM"))
        dram = ctx.enter_context(tc.tile_pool(name="ccdram", bufs=2, space="DRAM"))
        pools = {"pacc": pacc, "tp": tp}

        ident = const.tile([P, P], F32, name="ident")
        make_identity(nc, ident[:])
        ones_f = const.tile([P, P], BF16, name="ones_f")
        nc.vector.memset(ones_f[:], 1.0)
        ones_b = const.tile([P, HD], BF16, name="ones_b")
        nc.vector.memset(ones_b[:], 1.0)
        eps_t = const.tile([P, 1], F32, name="eps_t")
        nc.vector.memset(eps_t[:], EPS)

        # params -> [128, n] tiles
        def ldvec(dt_ap, n, name):
            t = const.tile([P, n], F32, tag=name, name=name)
            nc.sync.dma_start(out=t[:], in_=dt_ap.rearrange("(a p) -> p a", p=P))
            return t

        t_ln1g = [ldvec(d_ln1g.ap()[l], NE, f"ln1g{l}") for l in range(L)]
        t_ln1b = [ldvec(d_ln1b.ap()[l], NE, f"ln1b{l}") for l in range(L)]
        t_ln2g = [ldvec(d_ln2g.ap()[l], NE, f"ln2g{l}") for l in range(L)]
        t_ln2b = [ldvec(d_ln2b.ap()[l], NE, f"ln2b{l}") for l in range(L)]
        t_bo = [ldvec(d_bo.ap()[l], NE, f"bo{l}") for l in range(L)]
        t_b1 = [ldvec(d_b1.ap()[l], NFT, f"b1{l}") for l in range(L)]
        t_b2 = [ldvec(d_b2.ap()[l], NE, f"b2{l}") for l in range(L)]
        t_lnfg = ldvec(d_lnfg.ap(), NE, "lnfg")
        t_lnfb = ldvec(d_lnfb.ap(), NE, "lnfb")
        t_blm = ldvec(d_blm.ap(), NVT, "blm")

        # causal masks (bf16 multiplicative, per-core data)
        mask_t = []
        for sb in range(NSB):
            m = pp_msk.tile([P, S], BF16, tag=f"msk{sb}", name=f"msk{sb}")
            nc.sync.dma_start(out=m[:], in_=d_msk.ap()[sb])
            mask_t.append(m)

        # residual stream xT: 8 tiles [128, S]
        xT = [pp_x.tile([P, S], F32, tag=f"x{e}", name=f"x{e}") for e in range(NE)]
        sum_t = [pp_sum.tile([P, S], F32, tag=f"s{e}", name=f"s{e}") for e in range(NE)]

        # ---- embedding gather + transpose into xT ----
        idx_t = const.tile([P, S // P], I32, name="idx_t")
        nc.sync.dma_start(out=idx_t[:], in_=d_idx.ap().rearrange("(g p) -> p g", p=P))
        for g in range(S // P):
            xg = tp.tile([P, E], F32, tag="embg", name="embg", bufs=2)
            nc.gpsimd.indirect_dma_start(
                out=xg[:], out_offset=None, in_=d_emb.ap(),
                in_offset=bass.IndirectOffsetOnAxis(ap=idx_t[:, g:g + 1], axis=0))
            for e in range(NE):
                pst = pacc.tile([P, S], F32, tag="ps_a", name="tpose", bufs=6)
                nc.tensor.transpose(pst[:, 0:P], xg[:, e * P:(e + 1) * P], ident[:])
                nc.vector.tensor_copy(xT[e][:, g * P:(g + 1) * P], pst[:, 0:P])

        # ---- transformer blocks ----
        for l in range(L):
            # q projection (W-stationary): qT tiles [128, S]
            with tc.tile_pool(name="qT", bufs=1) as pp_q, \
                 tc.tile_pool(name="oT", bufs=1) as pp_o, \
                 tc.tile_pool(name="attn", bufs=1) as ap_t, \
                 tc.tile_pool(name="vslab", bufs=1) as vp:

                # k projection -> kT_local -> DRAM
                xA = []
                for e in range(NE):
                    xa = ap_t.tile([P, S], BF16, tag=f"xa{e}", name=f"xa{e}")
                    nc.vector.tensor_copy(xa[:], xT[e][:])
                    xA.append(xa)
                kvink = dram.tile([E, S], FP8, tag="kvink", name="kvink")
                kvoutk = dram.tile([CH * E, S], FP8, tag="kvoutk", name="kvoutk")
                for m in range(NE):
                    if m % 2 == 0:
                        wkp = wp.tile([P, 2, NE, P], BF16, tag="wb", name="wb", bufs=3)
                        nc.sync.dma_start(out=wkp[:], in_=d_wkr.ap()[l, m // 2])
                    ps = pacc.tile([P, S], F32, tag="ps_a", name="acc", bufs=6)
                    for k in range(NE):
                        nc.tensor.matmul(ps[:], lhsT=wkp[:, m % 2, k, :], rhs=xA[k][:],
                                         start=(k == 0), stop=(k == NE - 1))
                    kl = tp.tile([P, S], FP8, tag="klocal", name="klocal", bufs=2)
                    nc.vector.tensor_copy(kl[:], ps[:])
                    nc.sync.dma_start(out=kvink[m * P:(m + 1) * P, :], in_=kl[:])

                nc.gpsimd.collective_compute(
                    "AllGather", OP.bypass, replica_groups=groups,
                    ins=[kvink[:]], outs=[kvoutk[:]])

                # v projection (x-stationary) -> v_local bf16 -> DRAM
                kvinv = dram.tile([S, E], FP8, tag="kvinv", name="kvinv")
                for n in range(2):
                    wv = wp.tile([P, NE * 512], BF16, tag="wbig", name="wv", bufs=2)
                    nc.sync.dma_start(
                        out=wv[:].rearrange("p (a b) -> p a b", a=NE),
                        in_=d_wvf.ap()[l, n])
                    psv = [pacc.tile([P, 512], F32, tag="ps_a", name="acc", bufs=6) for _ in range(4)]
                    for k in range(NE):
                        for m in range(4):
                            nc.tensor.matmul(psv[m][:], lhsT=xA[k][:, m * P:(m + 1) * P],
                                             rhs=wv[:, k * 512:(k + 1) * 512],
                                             start=(k == 0), stop=(k == NE - 1))
                    for m in range(4):
                        vv = tp.tile([P, 512], FP8, tag="vlocal", name="vlocal", bufs=2)
                        nc.vector.tensor_copy(vv[:], psv[m][:])
                        nc.sync.dma_start(
                            out=kvinv[m * P:(m + 1) * P, n * 512:(n + 1) * 512], in_=vv[:])

                # group-local AllGather of kT and v (per-batch groups)
                kvoutv = dram.tile([T, E], FP8, tag="kvoutv", name="kvoutv")
                nc.gpsimd.collective_compute(
                    "AllGather", OP.bypass, replica_groups=groups,
                    ins=[kvinv[:]], outs=[kvoutv[:]])

                qT = []
                for m in range(NE):
                    if m % 2 == 0:
                        wqp = wp.tile([P, 2, NE, P], BF16, tag="wb", name="wb", bufs=3)
                        nc.sync.dma_start(out=wqp[:], in_=d_wqr.ap()[l, m // 2])
                    ps = pacc.tile([P, S], F32, tag="ps_a", name="acc", bufs=6)
                    for k in range(NE):
                        nc.tensor.matmul(ps[:], lhsT=wqp[:, m % 2, k, :], rhs=xA[k][:],
                                         start=(k == 0), stop=(k == NE - 1))
                    q = pp_q.tile([P, S], BF16, tag=f"q{m}", name=f"q{m}")
                    nc.vector.tensor_copy(q[:], ps[:])
                    qT.append(q)


                # attention, pair-major; all 16 key blocks on every core
                oT = []
                for p in range(NHP):
                    pav = pacc.tile([P, S], F32, tag="ps_b", name="pav", bufs=2)
                    prs = pacc.tile([P, S], F32, tag="ps_b", name="prs", bufs=2)
                    vsl = []
                    for sb in range(NSB):
                        v = vp.tile([P, P], FP8, tag="vsl", name="vsl", bufs=20)
                        nc.sync.dma_start(
                            out=v[:],
                            in_=kvoutv[sb * P:(sb + 1) * P, p * P:(p + 1) * P])
                        vsl.append(v)
                    for cc in range(CH):
                        ksl = ap_t.tile([P, S], FP8, tag="kslab", name="kslab", bufs=3)
                        nc.sync.dma_start(
                            out=ksl[:],
                            in_=kvoutk[cc * E + p * P:cc * E + (p + 1) * P, :])
                        for j2 in range(4):
                            sb = 4 * cc + j2
                            first, last = (sb == 0), (sb == NSB - 1)
                            sA = pacc.tile([P, S], F32, tag="ps_a", name="sc", bufs=6)
                            sB = pacc.tile([P, S], F32, tag="ps_a", name="sc", bufs=6)
                            nc.tensor.matmul(
                                sA[:], lhsT=ksl[0:HD, j2 * P:(j2 + 1) * P],
                                rhs=qT[p][0:HD, :], tile_position=(0, 0))
                            nc.tensor.matmul(
                                sB[:], lhsT=ksl[HD:P, j2 * P:(j2 + 1) * P],
                                rhs=qT[p][HD:P, :], tile_position=(64, 0))
                            pa = ap_t.tile([P, S], BF16, tag="pt", name="pt", bufs=4)
                            pb = ap_t.tile([P, S], BF16, tag="pt", name="pt", bufs=4)
                            nc.scalar.activation(pa[:], sA[:], AF.Exp, scale=HD ** -0.5)
                            nc.scalar.activation(pb[:], sB[:], AF.Exp, scale=HD ** -0.5)
                            nc.vector.tensor_tensor(out=pa[:], in0=pa[:],
                                                    in1=mask_t[sb][:], op=OP.mult)
                            nc.vector.tensor_tensor(out=pb[:], in0=pb[:],
                                                    in1=mask_t[sb][:], op=OP.mult)
                            nc.tensor.matmul(
                                pav[0:HD, :], lhsT=vsl[sb][:, 0:HD],
                                rhs=pa[:], start=first, stop=last,
                                tile_position=(0, 0), skip_group_check=True)
                            nc.tensor.matmul(
                                pav[HD:P, :], lhsT=vsl[sb][:, HD:P],
                                rhs=pb[:], start=first, stop=last,
                                tile_position=(0, 64), skip_group_check=True)
                            nc.tensor.matmul(
                                prs[0:HD, :], lhsT=ones_b[:], rhs=pa[:],
                                start=first, stop=last,
                                tile_position=(0, 0), skip_group_check=True)
                            nc.tensor.matmul(
                                prs[HD:P, :], lhsT=ones_b[:], rhs=pb[:],
                                start=first, stop=last,
                                tile_position=(0, 64), skip_group_check=True)
                    rec = tp.tile([P, S], F32, tag="rec", name="rec", bufs=2)
                    nc.vector.reciprocal(rec[:], prs[:])
                    o = pp_o.tile([P, S], BF16, tag=f"o{p}", name=f"o{p}")
                    nc.vector.tensor_tensor(out=o[:], in0=pav[:], in1=rec[:], op=OP.mult)
                    oT.append(o)

                # output projection + bias + residual
                for m in range(NE):
                    if m % 2 == 0:
                        wop = wp.tile([P, 2, NE, P], BF16, tag="wb", name="wb", bufs=3)
                        nc.sync.dma_start(out=wop[:], in_=d_wor.ap()[l, m // 2])
                    ps = pacc.tile([P, S], F32, tag="ps_a", name="acc", bufs=6)
                    for k in range(NE):
                        nc.tensor.matmul(ps[:], lhsT=wop[:, m % 2, k, :], rhs=oT[k][:],
                                         start=(k == 0), stop=(k == NE - 1))
                    yv = tp.tile([P, S], F32, tag="yv", name="yv", bufs=2)
                    nc.scalar.add(yv[:], ps[:], t_bo[l][:, m:m + 1])
                    nc.vector.tensor_tensor(out=sum_t[m][:], in0=yv[:], in1=xT[m][:],
                                            op=OP.add)

            _ln_tiles(nc, tc, pools, sum_t, xT, t_ln1g[l], t_ln1b[l], ones_f, eps_t)

            # FFN (bf16 matmuls, fp32 psum + residual)
            with tc.tile_pool(name="ht", bufs=1) as pp_h, \
                 tc.tile_pool(name="xbf", bufs=1) as pp_xbf:
                xF = []
                for e in range(NE):
                    xf = pp_xbf.tile([P, S], BF16, tag=f"xf{e}", name=f"xf{e}")
                    nc.vector.tensor_copy(xf[:], xT[e][:])
                    xF.append(xf)
                hT = []
                for f in range(NFT):
                    if f % 2 == 0:
                        w1p = wp.tile([P, 2, NE, P], BF16, tag="wb", name="wb", bufs=3)
                        nc.sync.dma_start(out=w1p[:], in_=d_w1r.ap()[l, f // 2])
                    ps = pacc.tile([P, S], F32, tag="ps_a", name="acc", bufs=6)
                    for k in range(NE):
                        nc.tensor.matmul(ps[:], lhsT=w1p[:, f % 2, k, :], rhs=xF[k][:],
                                         start=(k == 0), stop=(k == NE - 1))
                    h = pp_h.tile([P, S], BF16, tag=f"h{f}", name=f"h{f}")
                    nc.scalar.activation(h[:], ps[:], AF.Relu, bias=t_b1[l][:, f:f + 1])
                    hT.append(h)
                for m in range(NE):
                    w = wp.tile([P, NFT * P], BF16, tag="wbig", name="w2", bufs=2)
                    nc.sync.dma_start(
                        out=w[:].rearrange("p (a b) -> p a b", a=NFT),
                        in_=d_w2r.ap()[l, m])
                    ps = pacc.tile([P, S], F32, tag="ps_a", name="acc", bufs=6)
                    for f in range(NFT):
                        nc.tensor.matmul(ps[:], lhsT=w[:, f * P:(f + 1) * P],
                                         rhs=hT[f][:],
                                         start=(f == 0), stop=(f == NFT - 1))
                    yv = tp.tile([P, S], F32, tag="yv", name="yv", bufs=2)
                    nc.scalar.add(yv[:], ps[:], t_b2[l][:, m:m + 1])
                    nc.vector.tensor_tensor(out=sum_t[m][:], in0=yv[:], in1=xT[m][:],
                                            op=OP.add)
            _ln_tiles(nc, tc, pools, sum_t, xT, t_ln2g[l], t_ln2b[l], ones_f, eps_t)

        # ---- final LN + lm_head ----
        _ln_tiles(nc, tc, pools, xT, sum_t, t_lnfg, t_lnfb, ones_f, eps_t)
        with tc.tile_pool(name="lg", bufs=1) as pp_lg, \
             tc.tile_pool(name="xb", bufs=1) as pp_xb:
            xB = []
            for e in range(NE):
                xb = pp_xb.tile([P, S], BF16, tag=f"xb{e}", name=f"xb{e}")
                nc.vector.tensor_copy(xb[:], sum_t[e][:])
                xB.append(xb)
            for vt2 in range(0, NVT, 2):
                w = wp.tile([P, 2, NE, P], BF16, tag="wb", name="wlmt", bufs=3)
                nc.sync.dma_start(out=w[:], in_=d_wlm.ap()[vt2 // 2])
                for g in range(2):
                    vt = vt2 + g
                    ps = pacc.tile([P, S], F32, tag="ps_a", name="acc", bufs=6)
                    for k in range(NE):
                        nc.tensor.matmul(ps[:], lhsT=w[:, g, k, :], rhs=xB[k][:],
                                         start=(k == 0), stop=(k == NE - 1))
                    lg = pp_lg.tile([P, S], F32, tag="lg", name="lg", bufs=4)
                    nc.scalar.add(lg[:], ps[:], t_blm[:, vt:vt + 1])
                    nc.sync.dma_start(out=d_out.ap()[vt * P:(vt + 1) * P, :], in_=lg[:])

    nc.compile()
    return nc


_CACHED = {}


def _swz(w_me):
    m, e, p = w_me.shape
    return np.ascontiguousarray(
        w_me.reshape(m // 2, 2, NE, P, p).transpose(0, 3, 1, 2, 4))


def _prep_weights(inputs):
    f32 = np.float32
    bf = ml_dtypes.bfloat16
    Wq, Wk, Wv = inputs["Wq"], inputs["Wk"], inputs["Wv"]
    wq_flat = np.ascontiguousarray(Wq.transpose(0, 2, 1, 3).reshape(L, E, H * HD))
    wk_flat = np.ascontiguousarray(Wk.transpose(0, 2, 1, 3).reshape(L, E, H * HD))
    wv_flat = np.ascontiguousarray(Wv.transpose(0, 2, 1, 3).reshape(L, E, H * HD))
    wqr = np.stack([_swz(wq_flat[l].reshape(E, NE, P).transpose(1, 0, 2))
                    for l in range(L)]).astype(f32)
    wkr = np.stack([_swz(wk_flat[l].reshape(E, NE, P).transpose(1, 0, 2))
                    for l in range(L)]).astype(f32)
    wor = np.stack([_swz(inputs["Wo"][l].reshape(E, NE, P).transpose(1, 0, 2))
                    for l in range(L)]).astype(f32)
    w1r = np.stack([_swz(inputs["W1"][l].reshape(E, NFT, P).transpose(1, 0, 2))
                    for l in range(L)]).astype(f32)
    wvf = np.ascontiguousarray(
        wv_flat.reshape(L, NE, P, 2, 512).transpose(0, 3, 2, 1, 4)).astype(f32)
    w2r = np.ascontiguousarray(
        inputs["W2"].reshape(L, NFT, P, NE, P).transpose(0, 3, 2, 1, 4)).astype(f32)
    wlm_me = np.ascontiguousarray(
        inputs["Wlm"].reshape(E, NVT, P).transpose(1, 0, 2))
    wlmr = _swz(wlm_me).astype(f32)
    return {
        "wqr": wqr.astype(bf), "wkr": wkr.astype(bf), "wvf": wvf.astype(bf),
        "wor": wor.astype(bf),
        "w1r": w1r.astype(bf), "w2r": w2r.astype(bf), "wlmr": wlmr.astype(bf),
        "emb": np.ascontiguousarray(inputs["emb"]).astype(f32),
        "ln1g": np.ascontiguousarray(inputs["ln1_g"]).astype(f32),
        "ln1b": np.ascontiguousarray(inputs["ln1_b"]).astype(f32),
        "ln2g": np.ascontiguousarray(inputs["ln2_g"]).astype(f32),
        "ln2b": np.ascontiguousarray(inputs["ln2_b"]).astype(f32),
        "bo": np.ascontiguousarray(inputs["bo"]).astype(f32),
        "b1": np.ascontiguousarray(inputs["b1"]).astype(f32),
        "b2": np.ascontiguousarray(inputs["b2"]).astype(f32),
        "lnfg": np.ascontiguousarray(inputs["lnf_g"]).astype(f32),
        "lnfb": np.ascontiguousarray(inputs["lnf_b"]).astype(f32),
        "blm": np.ascontiguousarray(inputs["blm"]).astype(f32),
    }


def kernel(**inputs):
    if "nc" not in _CACHED:
        _CACHED["nc"] = build_program()
    nc = _CACHED["nc"]

    shared = _prep_weights(inputs)
    index = np.asarray(inputs["index"])

    # per-core causal masks and token ids
    jpos = np.arange(S)
    ipos = np.arange(P)
    in_maps = []
    for c in range(NC):
        b, j = c // CH, c % CH
        q0 = j * S
        m = np.zeros((NSB, P, S), np.float32)
        for sb in range(NSB):
            m[sb] = ((sb * P + ipos)[:, None] <= (q0 + jpos)[None, :])
        im = dict(shared)
        im["maskp"] = m.astype(ml_dtypes.bfloat16)
        im["idx"] = np.ascontiguousarray(index[b, q0:q0 + S]).astype(np.int32)
        in_maps.append(im)

    res = bass_utils.run_bass_kernel_spmd(nc, in_maps, core_ids=list(range(NC)))
    out = np.zeros((B, T, V), np.float32)
    for c in range(NC):
        b, j = c // CH, c % CH
        out[b, j * S:(j + 1) * S, :] = res.results[c]["logt"].T
    return out

